# revision 3
# baseline (speedup 1.0000x reference)
"""Multi-head attention (B=2, S=2048, D=1024, H=16, causal, unscaled scores)
on 8 Trainium2 NeuronCores.

Sharding: 2 batches x 4 head-groups (4 heads each). Core c handles batch
c//4, heads 4*(c%4) .. 4*(c%4)+3. Each core computes its group's QKV
projections, causal attention, and a partial output projection
(row-slice of wo); the host sums the 4 partials per batch (the
all-reduce) and adds the bias terms.

Schedule: the three phases are interleaved at chunk granularity so the
PE and Act engines overlap. Attention on i-slice IS runs with PE
"filler" chunks — the QKV projection chains for m-slice IS+1 and the
output-projection tiles for i-slice IS-1 — emitted between attention
units, so PE computes projections while Act runs the softmax exp.
Projections are 8-matmul K-chains over per-k-tile x tiles streamed
through rotating pools (dedicated tag per input, depth 10), giving the
DMA a full m-slice of prefetch headroom without SBUF blowup.

Device layout avoids all on-chip transposes:
  - host passes q/k/v transposed ([D, S]) so projections produce
    QHT/KHT = (x@w).T with head-dim on partitions (score-ready)
  - VH is produced in natural [S, D_head] orientation with an extra
    ones column, so the attnV matmul also accumulates the softmax
    denominator (row 64 of U^T)
  - normalization is deferred: U^T is copied out raw (freeing its PSUM
    bank), then per head-pair two accumulating K=1 selector-row matmuls
    broadcast both reciprocal rows into one [128,512] bank and a single
    full-width multiply rescales ct in place; the bias terms bv/bo are
    folded in exactly on the host (C = U/colsum + 1*bv since softmax
    rows sum to 1).
All matmuls run as float32r (bf16-pair fp32: ~1e-4 rel err, 4x the
throughput of plain fp32).
"""

import numpy as np

D = 1024
S = 2048
NH = 16
DH = 64
B = 2
G = 4            # head-groups = cores per batch
HG = NH // G     # 4 heads per group
GD = HG * DH     # 256 columns per group
KT = D // 128    # 8 k-tiles
MS = S // 512    # 4 m-slices
JT = S // 128    # 16 j-tiles
IST = S // 512   # 4 i-slices

_cached = None

_SEL = np.zeros((2, 128), np.float32)
_SEL[0, 0:64] = 1.0
_SEL[1, 64:128] = 1.0


def _build():
    from concourse import bacc
    import concourse.mybir as mybir
    import concourse.tile as tile

    f32 = mybir.dt.float32
    f32r = mybir.dt.float32r
    Act = mybir.ActivationFunctionType
    Alu = mybir.AluOpType

    nc = bacc.Bacc(None, target_bir_lowering=False)
    xq = nc.dram_tensor("xq", [D, S], f32r, kind="ExternalInput")
    xk = nc.dram_tensor("xk", [D, S], f32r, kind="ExternalInput")
    xv = nc.dram_tensor("xv", [D, S], f32r, kind="ExternalInput")
    wqg = nc.dram_tensor("wqg", [D, GD], f32r, kind="ExternalInput")
    wkg = nc.dram_tensor("wkg", [D, GD], f32r, kind="ExternalInput")
    wvg = nc.dram_tensor("wvg", [D, GD], f32r, kind="ExternalInput")
    wog = nc.dram_tensor("wog", [GD, D], f32r, kind="ExternalInput")
    bqg = nc.dram_tensor("bqg", [2, 128, 1], f32, kind="ExternalInput")
    bkg = nc.dram_tensor("bkg", [2, 128, 1], f32, kind="ExternalInput")
    selg = nc.dram_tensor("selg", [2, 128], f32r, kind="ExternalInput")
    outp = nc.dram_tensor("outp", [S, D], f32, kind="ExternalOutput")
    xsrc = {"q": xq, "k": xk, "v": xv}

    with tile.TileContext(nc) as tc:
        with (
            tc.tile_pool(name="wpool", bufs=1) as wpool,
            tc.tile_pool(name="xstr", bufs=10) as xstr,
            tc.tile_pool(name="big", bufs=1) as big,
            tc.tile_pool(name="ppool", bufs=6) as ppool,
            tc.tile_pool(name="small", bufs=6) as small,
            tc.tile_pool(name="osb", bufs=3) as osb,
            tc.tile_pool(name="ps", bufs=2, space="PSUM") as ps,
            tc.tile_pool(name="po", bufs=2, space="PSUM") as po,
            tc.tile_pool(name="psU", bufs=2, space="PSUM") as psU,
        ):
            # ---- resident weights / constants ----
            wq_t = wpool.tile([128, KT, GD], f32r, tag="wq")
            wk_t = wpool.tile([128, KT, GD], f32r, tag="wk")
            wv_t = wpool.tile([128, KT, GD], f32r, tag="wv")
            wo_t = wpool.tile([128, 2, D], f32r, tag="wo")
            bq_t = wpool.tile([128, 2, 1], f32, tag="bq")
            bk_t = wpool.tile([128, 2, 1], f32, tag="bk")
            sel0 = wpool.tile([1, 128], f32r, tag="sel0")
            sel1 = wpool.tile([1, 128], f32r, tag="sel1")
            wt = {"q": wq_t, "k": wk_t, "v": wv_t}
            bt = {"q": bq_t, "k": bk_t}
            wql = xq_like_w(wqg)
            nc.sync.dma_start(out=wq_t[:, 0:2, :], in_=wql[:, 0:2, :])
            nc.sync.dma_start(out=wq_t[:, 2:KT, :], in_=wql[:, 2:KT, :])
            nc.sync.dma_start(out=bq_t, in_=bqg[:].rearrange("t p o -> p t o"))
            nc.sync.dma_start(out=bk_t, in_=bkg[:].rearrange("t p o -> p t o"))
            nc.sync.dma_start(out=sel0, in_=selg[0:1, :])
            nc.sync.dma_start(out=sel1, in_=selg[1:2, :])

            qht = big.tile([128, 2, S], f32r, tag="qht")
            kht = big.tile([128, 2, S], f32r, tag="kht")
            vh = big.tile([128, JT, HG, DH + 1], f32r, tag="vh")
            ct = big.tile([128, 2, S], f32r, tag="ct")
            dst = {"q": qht, "k": kht}
            vh_ones_stage = wpool.tile([128, JT, HG, 1], f32, tag="vh_ones_st")
            nc.vector.memset(vh_ones_stage, 1.0)
            nc.scalar.activation(
                out=vh[:, :, :, DH : DH + 1], in_=vh_ones_stage, func=Act.Copy
            )

            # ---- projection building blocks ----
            xtiles = {}

            def emit_x_dma(which, m, kk):
                xt = xstr.tile(
                    [128, 512], f32r, tag=f"x{which}", name=f"x{which}t"
                )
                nc.sync.dma_start(
                    out=xt,
                    in_=xsrc[which][kk * 128 : (kk + 1) * 128, m * 512 : (m + 1) * 512],
                )
                xtiles[(which, m, kk)] = xt

            def chunk_qk(which, m, n):
                psum = po.tile([128, 512], f32, tag="po", name="pp")
                w_t = wt[which]
                for kk in range(KT):
                    nc.tensor.matmul(
                        psum,
                        w_t[:, kk, n * 128 : (n + 1) * 128],
                        xtiles[(which, m, kk)],
                        start=(kk == 0),
                        stop=(kk == KT - 1),
                    )
                nc.vector.tensor_scalar_add(
                    dst[which][:, n, m * 512 : (m + 1) * 512], psum, bt[which][:, n, :]
                )

            def chunk_v(m, jj):
                psum = po.tile([128, GD], f32, tag="po", name="pv")
                for kk in range(KT):
                    nc.tensor.matmul(
                        psum,
                        xtiles[("v", m, kk)][:, jj * 128 : (jj + 1) * 128],
                        wv_t[:, kk, :],
                        start=(kk == 0),
                        stop=(kk == KT - 1),
                    )
                nc.vector.tensor_copy(
                    vh[:, m * 4 + jj, :, 0:DH],
                    psum[:].rearrange("p (h d) -> p h d", h=HG),
                )

            def proj_chunks(m):
                return [
                    lambda m=m: chunk_qk("q", m, 0),
                    lambda m=m: chunk_qk("q", m, 1),
                    lambda m=m: chunk_qk("k", m, 0),
                    lambda m=m: chunk_qk("k", m, 1),
                    lambda m=m: chunk_v(m, 0),
                    lambda m=m: chunk_v(m, 1),
                    lambda m=m: chunk_v(m, 2),
                    lambda m=m: chunk_v(m, 3),
                ]

            # ---- output projection building blocks ----
            out_sbs = {}

            def chunk_outproj(IS, it, nn):
                r0 = IS * 512 + it * 128
                if nn == 0:
                    out_sb = osb.tile([128, D], f32, tag="out")
                    out_sbs[(IS, it)] = out_sb
                else:
                    out_sb = out_sbs[(IS, it)]
                o_psum = po.tile([128, 512], f32, tag="po", name="po2")
                for t in range(2):
                    nc.tensor.matmul(
                        o_psum,
                        ct[:, t, r0 : r0 + 128],
                        wo_t[:, t, nn * 512 : (nn + 1) * 512],
                        start=(t == 0),
                        stop=(t == 1),
                    )
                nc.vector.tensor_copy(out_sb[:, nn * 512 : (nn + 1) * 512], o_psum)
                if nn == 1:
                    nc.sync.dma_start(out=outp[r0 : r0 + 128, :], in_=out_sb)

            def outproj_chunks(IS):
                return [
                    (lambda IS=IS, it=it, nn=nn: chunk_outproj(IS, it, nn))
                    for it in range(4)
                    for nn in range(2)
                ]

            # ---- m=0 projections (startup, no attention to overlap) ----
            for kk in range(KT):
                emit_x_dma("q", 0, kk)
            nc.sync.dma_start(out=wk_t, in_=xq_like_w(wkg))
            for kk in range(KT):
                emit_x_dma("k", 0, kk)
            nc.sync.dma_start(out=wv_t, in_=xq_like_w(wvg))
            for kk in range(KT):
                emit_x_dma("v", 0, kk)
            nc.sync.dma_start(
                out=wo_t, in_=wog[:].rearrange("(t p) n -> p t n", p=128)
            )
            for ch in proj_chunks(0):
                ch()

            # ---- attention with interleaved filler chunks ----
            def emit_attention(IS, fillers):
                i0 = IS * 512
                n_j = (IS + 1) * 4
                recips = {}
                n_units_total = sum(
                    len(range((n_j - 4) // 2)) + 4 for _ in range(HG // 2)
                )
                emitted_units = 0

                def filler():
                    # keep filler supply spread across remaining units
                    remaining = n_units_total - emitted_units
                    want = 2 if len(fillers) > remaining else 1
                    for _ in range(want):
                        if fillers:
                            fillers.popleft()()

                for hp in range(HG // 2):
                    nt = hp
                    u_psums = [
                        psU.tile([128, 512], f32, tag="u", name=f"u{e}")
                        for e in range(2)
                    ]
                    n_full = n_j - 4
                    units = []
                    for Jg in range(n_full // 2):
                        units.append(("full", Jg))
                    for J in range(n_full, n_j):
                        units.append(("diag", J))
                    pts = {}
                    s_psums = {}

                    def emit_scores(u):
                        kind, idx = u
                        if kind == "full":
                            for e in range(2):
                                lo = 64 * e
                                s_psum = ps.tile([128, 2, 512], f32, tag="ps")
                                for half in range(2):
                                    J = 2 * idx + half
                                    nc.tensor.matmul(
                                        s_psum[:, half, :],
                                        kht[lo : lo + DH, nt, J * 128 : (J + 1) * 128],
                                        qht[lo : lo + DH, nt, i0 : i0 + 512],
                                        start=True,
                                        stop=True,
                                    )
                                s_psums[(e, u)] = s_psum
                        else:
                            J = idx
                            r = J * 128 - i0
                            s_psum = ps.tile([128, 2, 512], f32, tag="ps", name="sd")
                            for e in range(2):
                                lo = 64 * e
                                nc.tensor.matmul(
                                    s_psum[:, e, 0 : 512 - r],
                                    kht[lo : lo + DH, nt, J * 128 : (J + 1) * 128],
                                    qht[lo : lo + DH, nt, i0 + r : i0 + 512],
                                    start=True,
                                    stop=True,
                                )
                            s_psums[(0, u)] = s_psum

                    def emit_exp_mask(u):
                        kind, idx = u
                        if kind == "full":
                            for e in range(2):
                                pt = ppool.tile([128, 2, 512], f32r, tag="pt")
                                nc.scalar.activation(
                                    out=pt, in_=s_psums[(e, u)], func=Act.Exp
                                )
                                pts[(e, u)] = pt
                        else:
                            r = idx * 128 - i0
                            w = 512 - r
                            pt = ppool.tile([128, 2, 512], f32r, tag="pt", name="ptd")
                            nc.scalar.activation(
                                out=pt[:, :, 0:w],
                                in_=s_psums[(0, u)][:, :, 0:w],
                                func=Act.Exp,
                            )
                            nc.gpsimd.affine_select(
                                out=pt[:, :, 0:w],
                                in_=pt[:, :, 0:w],
                                compare_op=Alu.is_ge,
                                fill=0.0,
                                base=0,
                                pattern=[[0, 2], [1, w]],
                                channel_multiplier=-1,
                            )
                            pts[(0, u)] = pt

                    def emit_attnv(u):
                        kind, idx = u
                        for e in range(2):
                            if kind == "full":
                                for half in range(2):
                                    J = 2 * idx + half
                                    nc.tensor.matmul(
                                        u_psums[e][0 : DH + 1, :],
                                        vh[:, J, 2 * hp + e, :],
                                        pts[(e, u)][:, half, :],
                                        start=(J == 0),
                                        stop=False,
                                    )
                            else:
                                J = idx
                                r = J * 128 - i0
                                nc.tensor.matmul(
                                    u_psums[e][0 : DH + 1, r:512],
                                    vh[:, J, 2 * hp + e, :],
                                    pts[(0, u)][:, e, 0 : 512 - r],
                                    start=(J == 0),
                                    stop=(J == n_j - 1),
                                )

                    # software pipeline: scores 1 unit ahead of attnV, with
                    # PE filler between exp and the previous unit's attnV
                    emit_scores(units[0])
                    emit_exp_mask(units[0])
                    for ui in range(1, len(units)):
                        emit_scores(units[ui])
                        emit_exp_mask(units[ui])
                        filler()
                        emitted_units += 1
                        emit_attnv(units[ui - 1])
                    emit_attnv(units[-1])
                    filler()
                    emitted_units += 1

                    for e in range(2):
                        lo = 64 * e
                        recip = small.tile([1, 512], f32r, tag="recip", name=f"rc{e}")
                        with nc.allow_low_precision(reason="fp32r is fp32-width"):
                            nc.vector.reciprocal(recip, u_psums[e][DH : DH + 1, :])
                        nc.vector.tensor_copy(
                            ct[lo : lo + DH, nt, i0 : i0 + 512], u_psums[e][0:DH, :]
                        )
                        recips[(hp, e)] = recip
                    if hp > 0:
                        emit_normalize_pair(IS, hp - 1, recips)
                emit_normalize_pair(IS, HG // 2 - 1, recips)
                # drain any leftover fillers
                while fillers:
                    fillers.popleft()()

            def emit_normalize_pair(IS, hp, recips):
                i0 = IS * 512
                bc_psum = ps.tile([128, 512], f32, tag="ps", name="bcp")
                for e, sel in ((0, sel0), (1, sel1)):
                    nc.tensor.matmul(
                        bc_psum, sel, recips[(hp, e)], start=(e == 0), stop=(e == 1)
                    )
                nc.vector.tensor_mul(
                    ct[:, hp, i0 : i0 + 512],
                    bc_psum,
                    ct[:, hp, i0 : i0 + 512],
                )

            from collections import deque

            for IS in range(IST):
                fillers = deque()
                if IS + 1 < MS:
                    # x DMAs for the next m-slice land while this slice runs
                    for which in ("q", "k", "v"):
                        for kk in range(KT):
                            emit_x_dma(which, IS + 1, kk)
                    pcs = proj_chunks(IS + 1)
                else:
                    pcs = []
                ocs = outproj_chunks(IS - 1) if IS > 0 else []
                # outproj first (its ct inputs are ready; proj chunks wait on
                # the x DMAs just issued), proj chains are ~4x bigger
                pi = oi = 0
                while pi < len(pcs) or oi < len(ocs):
                    for _ in range(2):
                        if oi < len(ocs):
                            fillers.append(ocs[oi])
                            oi += 1
                    if pi < len(pcs):
                        fillers.append(pcs[pi])
                        pi += 1
                emit_attention(IS, fillers)
            for ch in outproj_chunks(IST - 1):
                ch()

    nc.compile()
    return nc


def xq_like_w(w):
    return w[:].rearrange("(kt p) n -> p kt n", p=128)


def _get_nc():
    global _cached
    if _cached is None:
        _cached = _build()
    return _cached


def _in_maps(q, k, v, wq, bq, wk, bk, wv, bv, wo, bo):
    maps = []
    for c in range(8):
        b, g = c // G, c % G
        cs = slice(g * GD, (g + 1) * GD)
        maps.append(
            {
                "xq": np.ascontiguousarray(q[b].T).astype(np.float32, copy=False),
                "xk": np.ascontiguousarray(k[b].T).astype(np.float32, copy=False),
                "xv": np.ascontiguousarray(v[b].T).astype(np.float32, copy=False),
                "wqg": np.ascontiguousarray(wq[:, cs]),
                "wkg": np.ascontiguousarray(wk[:, cs]),
                "wvg": np.ascontiguousarray(wv[:, cs]),
                "wog": np.ascontiguousarray(wo[cs, :]),
                "bqg": np.ascontiguousarray(bq[cs]).reshape(2, 128, 1),
                "bkg": np.ascontiguousarray(bk[cs]).reshape(2, 128, 1),
                "selg": _SEL,
            }
        )
    return maps


def run(inputs, trace=False, trace_kwargs=None):
    from concourse.bass_utils import run_bass_kernel_spmd

    nc = _get_nc()
    maps = _in_maps(**inputs)
    res = run_bass_kernel_spmd(
        nc, maps, list(range(8)), trace=trace, **(trace_kwargs or {})
    )
    out = np.zeros((B, S, D), np.float32)
    for c in range(8):
        out[c // G] += res.results[c]["outp"]
    # exact bias fold: C = U/colsum + 1 (x) bv  =>  out += bv @ wo + bo
    out += inputs["bv"].astype(np.float32) @ inputs["wo"].astype(np.float32)
    out += inputs["bo"].astype(np.float32)
    return out.astype(np.float32), res


def kernel(**inputs) -> np.ndarray:
    out, _ = run(inputs)
    return out
